# revision 23
# baseline (speedup 1.0000x reference)
"""Bass/Trainium2 kernel for masked dot-product attention.

Math (per batch b):
  scores = q @ k^T / sqrt(D); masked positions (j >= valid[i]) -> 1e-6
  weights = softmax(scores, -1); out = weights @ v

Strategy (v3):
  - Shard batch dim B=16 across 8 cores (2 batches/core), SPMD program.
  - Host: sort rows of each batch by valid[i] -> monotone mask staircase;
    fully-masked (i,j)-tiles are skipped; their exact contribution
    exp(1e-6)*(suffix sums of v) is added via a host-built correction
    tensor (identity matmul into the same accumulator).
  - Device: S^T tiles [j=128 part, i<=512 free] on PE in fp16, with q
    pre-scaled by sqrt(A), k by sqrt(A) (A = 1024*0.125*log2 e) and a
    65th contraction row adding B = 15360 = 1024*15: the psum value IS
    the fp16 bit pattern of e^{s/8} (Schraudolph).  exp splits between:
      * ACT: exact exp (scale/bias fold A,B away), fp16 out
      * DVE: convert-to-int16 (round-to-nearest) + a 3-op quadratic
        mantissa correction, all in 2x/4x-eligible 16-bit ops.
    The mask staircase is a {1,0} fp16 multiply (fused into the DVE
    path's first op; a separate cheap multiply after ACT tiles).
  - AV: out[i-subtile 128, 65] accumulates per j-tile with E stationary
    (moving dim 65 incl. ones column -> softmax denominator for free).
  - Normalize: DVE reciprocal of z + broadcast multiply -> fp16 out.
  - GPSIMD cannot touch PSUM; it only zero-fills e-tile gap rows.
  - PE p-state warmup: 8 throwaway matmuls during the input-DMA window.
"""

import numpy as np

import concourse.bass as bass
import concourse.tile as tile
import concourse.mybir as mybir
from concourse import bacc
from concourse.bass_utils import run_bass_kernel_spmd
from concourse.masks import make_identity

B, N, D = 16, 2048, 64
NCORES = 8
NB = B // NCORES          # batches per core
IW = 512                  # i-range width
NI = N // IW              # 4 i-ranges
JW = 128                  # j tile width
NJ = N // JW              # 16 j tiles
SUB = 128                 # AV i-subtile
DV = D + 1

f16 = mybir.dt.float16
f32 = mybir.dt.float32
i16 = mybir.dt.int16

A_TRICK = 184.664955          # 1024 * 0.125 * log2(e)
SQ_A = float(np.sqrt(A_TRICK))
B_OFF = 15360.0               # 1024 * 15 (fp16 exponent bias)
SC_ACT = 0.125 / A_TRICK
E6 = float(np.exp(np.float32(1e-6)))

# cotrick: the product eh*eh2 (eh2 = bitcast(30720-bits)) equals the
# interp-error hump h(f) = (1+f)(2-f)/2; a linear map a+b*h approximates
# the exact correction c(f) = 2^f/(1+f) to +-0.6%. No common factor.
_ff = np.linspace(0.0, 1.0, 2001)
_cf = 2.0**_ff / (1.0 + _ff)
_hf = (1.0 + _ff) * (2.0 - _ff) / 2.0
CT_B, CT_A = [float(x) for x in np.polyfit(_hf, _cf, 1)]
LN_C2 = 0.0

LOOKAHEAD = 3
ACT_W = 0.75


class Plan:
    def __init__(self):
        self.taus = [[] for _ in range(NI)]   # per r: list of tau dicts
        self.m16_w = 0


def _classify(t_sorted):
    plan = Plan()
    off = 0
    for r in range(NI):
        tw = t_sorted[:, r * IW:(r + 1) * IW]  # [B, IW] sorted ascending
        for tau in range(NJ):
            jlo, jhi = JW * tau, JW * (tau + 1)
            n_le = (tw <= jlo).sum(axis=1)
            n_lt = (tw < jhi).sum(axis=1)
            lo = int(n_le.min())
            if lo >= IW:
                break
            mhi = int(n_lt.max())
            x0a = lo & ~127          # AV-subtile / S / exp start
            x0s = x0a
            w1 = max(mhi, x0s)
            ti = {
                "tau": tau, "lo": lo, "x0s": x0s, "x0a": x0a, "w1": w1,
                "m_off": None,
            }
            if w1 > x0s:
                ti["m_off"] = off
                off += w1 - x0s
            plan.taus[r].append(ti)
    plan.m16_w = max(off, 16)
    plan.m16_r = []
    for r in range(NI):
        offs = [ti["m_off"] for ti in plan.taus[r] if ti["m_off"] is not None]
        ws = [ti["w1"] - ti["x0s"] for ti in plan.taus[r]
              if ti["m_off"] is not None]
        if offs:
            plan.m16_r.append((min(offs), max(o + w for o, w in zip(offs, ws))))
        else:
            plan.m16_r.append(None)
    return plan


class _Greedy:
    """Pair-level balance between ACT (exact exp), DVE (cotrick) and
    GPSIMD (final multiply offload, SBUF-only)."""

    def __init__(self, act_w=1.0):
        self.load = {"ACT": 2600.0, "DVE": 600.0, "POOL": 800.0}
        self.act_w = act_w

    @staticmethod
    def c_act1(n):
        return (n + 222) * 0.8333

    @staticmethod
    def c_act2(n):
        return (2 * n + 222) * 0.8333

    @staticmethod
    def c_dve1(n, pool_ett):
        c = ((n + 120) + n + 3 * 58) * 1.0417
        if not pool_ett:
            c += (0.5 * n + 58) * 1.0417
        return c

    @staticmethod
    def c_dve2(n, clean, pool_ett):
        op1 = (2 * n + 120) if clean else 2 * (n + 120)
        c = (op1 + 2.0 * n + 3 * 58) * 1.0417
        if not pool_ett:
            c += (n + 58) * 1.0417
        return c

    @staticmethod
    def c_pool_ett(n2):
        return (n2 / 0.42) * 0.8333 + 95.0 + 1e9  # disabled: latency hurts

    def _mx(self, ca, cd, cp):
        return max(self.load["ACT"] + ca, self.load["DVE"] + cd,
                   self.load["POOL"] + cp)

    def pick_pair(self, ta, tb):
        """Returns (kind, pool_ett)."""
        na = IW - ta["x0s"]
        wba = (ta["w1"] - ta["x0s"])
        mska = (0.5 * wba + 58) * 1.0417 if wba else 0.0
        if tb is None:
            opts = {
                ("AA", False): (self.c_act1(na) * self.act_w, mska, 0.0),
                ("DD", False): (0.0, self.c_dve1(na, False), 0.0),
                ("DD", True): (0.0, self.c_dve1(na, True),
                               self.c_pool_ett(na)),
            }
        else:
            nb = IW - tb["x0s"]
            wbb = (tb["w1"] - tb["x0s"])
            mskb = (0.5 * wbb + 58) * 1.0417 if wbb else 0.0
            nm = IW - min(ta["x0s"], tb["x0s"])
            clean = (wba == 0 and wbb == 0 and ta["x0s"] == tb["x0s"])
            opts = {
                ("AA", False): (self.c_act2(nm) * self.act_w,
                                mska + mskb, 0.0),
                ("DD", False): (0.0, self.c_dve2(nm, clean, False), 0.0),
                ("DD", True): (0.0, self.c_dve2(nm, clean, True),
                               self.c_pool_ett(2 * nm)),
                ("AD", False): (self.c_act1(na) * self.act_w,
                                mska + self.c_dve1(nb, False), 0.0),
                ("DA", False): (self.c_act1(nb) * self.act_w,
                                mskb + self.c_dve1(na, False), 0.0),
            }
        best, bc = None, None
        for key, (ca, cd, cp) in opts.items():
            m = self._mx(ca, cd, cp)
            if bc is None or m < bc:
                best, bc = key, m
        ca, cd, cp = opts[best]
        self.load["ACT"] += ca
        self.load["DVE"] += cd
        self.load["POOL"] += cp
        return best

    def pick_norm(self):
        self.load["DVE"] += 129.0
        act_c = 4 * (64 + 222) * 0.8333 * self.act_w
        dve_c = (256 + 120) * 1.0417
        if self.load["ACT"] + act_c <= self.load["DVE"] + dve_c:
            self.load["ACT"] += act_c
            return "ACT"
        self.load["DVE"] += dve_c
        return "DVE"


def _build_program(plan):
    nc = bacc.Bacc("TRN2", target_bir_lowering=False, debug=False)

    qT = nc.dram_tensor("qT", [NB, DV, N], f16, kind="ExternalInput").ap()
    kT = nc.dram_tensor("kT", [NB, DV, N], f16, kind="ExternalInput").ap()
    vw = nc.dram_tensor("vw", [NB, 128, NJ, DV], f16, kind="ExternalInput").ap()
    corr = nc.dram_tensor("corr", [NB, 128, NI, 4, DV], f16,
                          kind="ExternalInput").ap()
    m16 = nc.dram_tensor("m16", [NB, 128, plan.m16_w], f16,
                         kind="ExternalInput").ap()
    out = nc.dram_tensor("out", [NB, 128, NI, 4, D], f16,
                         kind="ExternalOutput").ap()

    gr = _Greedy(act_w=ACT_W)

    with tile.TileContext(nc, trace_sim=False) as tc:
        with (
            tc.tile_pool(name="consts", bufs=1) as consts,
            tc.tile_pool(name="sb_qk", bufs=2) as sb_qk,
            tc.tile_pool(name="sb_vc", bufs=2) as sb_vc,
            tc.tile_pool(name="sb_e", bufs=8) as sb_e,
            tc.tile_pool(name="sb_eh", bufs=5) as sb_eh,
            tc.tile_pool(name="sb_t", bufs=5) as sb_t,
            tc.tile_pool(name="sb_o", bufs=3) as sb_o,
            tc.tile_pool(name="sb_z", bufs=3) as sb_z,
            tc.tile_pool(name="ps_pool", bufs=3, space="PSUM") as ps_pool,
            tc.tile_pool(name="ps_acc", bufs=2, space="PSUM") as ps_acc,
        ):
            ident = consts.tile([128, 128], f32)
            make_identity(nc, ident)
            identh = consts.tile([128, 128], f16)
            nc.vector.tensor_copy(identh, ident)
            bias_t = consts.tile([128, 1], f32)
            nc.vector.memset(bias_t, float(-B_OFF * SC_ACT + LN_C2))
            wsrc = consts.tile([128, 512], f16)
            nc.vector.memset(wsrc, 0.125)

            # p-state warmup (small moving dim: don't delay first S)
            for wu in range(6):
                ps_w = ps_pool.tile([128, 2, 512], f32, tag="s",
                                    name=f"wu{wu}")
                nc.tensor.matmul(ps_w[:, 0, 0:128], identh,
                                 wsrc[:, 0:128], start=True, stop=True)

            # input DMAs for both batches up-front (sync/SP queue)
            ins = []
            for bi in range(NB):
                q_sb = sb_qk.tile([DV, N], f16, tag="q")
                k_sb = sb_qk.tile([DV, N], f16, tag="k")
                m_sb = sb_vc.tile([128, plan.m16_w], f16, tag="m16")
                nc.sync.dma_start(out=q_sb, in_=qT[bi])
                nc.sync.dma_start(out=k_sb[:, 0:N // 2],
                                  in_=kT[bi][:, 0:N // 2])
                nc.sync.dma_start(out=k_sb[:, N // 2:],
                                  in_=kT[bi][:, N // 2:])
                corr_sb = sb_vc.tile([128, NI, 4, DV], f16, tag="corr")
                nc.sync.dma_start(out=corr_sb, in_=corr[bi])
                vw_sb = sb_vc.tile([128, NJ, DV], f16, tag="vw")
                nc.sync.dma_start(out=vw_sb, in_=vw[bi])
                nc.sync.dma_start(out=m_sb, in_=m16[bi])
                ins.append((q_sb, k_sb, vw_sb, corr_sb, m_sb))

            RING = 6

            def trick_op1(dst_i16, ps_ap, ti, m_sb):
                """Schraudolph op1 for one tau half into dst (int16 view)."""
                x0, w1 = ti["x0s"], ti["w1"]
                if w1 > x0:
                    mo = ti["m_off"]
                    nc.vector.scalar_tensor_tensor(
                        out=dst_i16, in0=ps_ap, scalar=0.0,
                        in1=m_sb[:, mo:mo + (IW - x0)],
                        op0=mybir.AluOpType.max, op1=mybir.AluOpType.mult)
                else:
                    nc.vector.tensor_scalar(
                        out=dst_i16, in0=ps_ap, scalar1=0.0, scalar2=None,
                        op0=mybir.AluOpType.max)

            def fix_chain(eh_ap, e_ap, nm, pool_ett=False):
                """cotrick fix: e = (CT_A + CT_B*(eh*eh2))*eh."""
                fb = sb_t.tile(list(eh_ap.shape), f16, tag="fb",
                               name=f"fb_{nm}")
                nc.vector.tensor_scalar(
                    out=fb.bitcast(i16), in0=eh_ap.bitcast(i16),
                    scalar1=-1.0, scalar2=30720.0,
                    op0=mybir.AluOpType.mult, op1=mybir.AluOpType.add)
                nc.vector.tensor_tensor(
                    out=fb, in0=eh_ap, in1=fb, op=mybir.AluOpType.mult)
                nc.vector.tensor_scalar(
                    out=fb, in0=fb, scalar1=float(CT_B), scalar2=float(CT_A),
                    op0=mybir.AluOpType.mult, op1=mybir.AluOpType.add)
                eng = nc.gpsimd if pool_ett else nc.vector
                eng.tensor_tensor(
                    out=e_ap, in0=fb, in1=eh_ap, op=mybir.AluOpType.mult)

            def act_exp(e_ap, ps_ap):
                nc.scalar.activation(
                    e_ap, ps_ap, mybir.ActivationFunctionType.Exp,
                    bias=bias_t[:, 0:1], scale=float(SC_ACT))

            def act_mask(e_t_ap, ti, m_sb):
                x0, w1 = ti["x0s"], ti["w1"]
                if w1 > x0:
                    mo = ti["m_off"]
                    nc.vector.tensor_tensor(
                        out=e_t_ap[:, x0:w1], in0=e_t_ap[:, x0:w1],
                        in1=m_sb[:, mo:mo + (w1 - x0)],
                        op=mybir.AluOpType.mult)

            pending_norm = []

            def emit_norm(bi3, r3, pacc3):
                zinv = sb_z.tile([128, 4], f32, tag="z",
                                 name=f"z_{bi3}_{r3}")
                nc.vector.reciprocal(zinv, pacc3[:, :, D])
                osb = sb_o.tile([128, 4, D], f16, tag="o",
                                name=f"o_{bi3}_{r3}")
                if gr.pick_norm() == "ACT":
                    for ib2 in range(4):
                        nc.scalar.activation(
                            osb[:, ib2, :], pacc3[:, ib2, 0:D],
                            mybir.ActivationFunctionType.Copy,
                            scale=zinv[:, ib2:ib2 + 1])
                else:
                    nc.vector.tensor_tensor(
                        out=osb, in0=pacc3[:, :, 0:D],
                        in1=zinv.unsqueeze(2).broadcast_to([128, 4, D]),
                        op=mybir.AluOpType.mult)
                nc.sync.dma_start(out=out[bi3][:, r3], in_=osb)

            for bi in range(NB):
                q_sb, k_sb, vw_sb, corr_sb, m_sb = ins[bi]
                for r in range(NI - 1, -1, -1):
                    taus = plan.taus[r]
                    pacc = ps_acc.tile([128, 4, DV], f32, tag="acc",
                                       name=f"acc_{bi}_{r}")
                    nc.tensor.matmul(
                        pacc, identh, corr_sb[:, r],
                        start=True, stop=False)

                    n_av = sum(4 - ti["x0a"] // SUB for ti in taus)
                    av_done = 0

                    pairs = [(i, i + 1 if i + 1 < len(taus) else None)
                             for i in range(0, len(taus), 2)]
                    ps_pairs = [None] * len(pairs)
                    e_pairs = [None] * len(pairs)

                    def emit_av_pair(pi2):
                        nonlocal av_done
                        ia2, ib2 = pairs[pi2]
                        for h2, idx2 in enumerate(
                                (ia2,) + ((ib2,) if ib2 is not None else ())):
                            ti2 = taus[idx2]
                            for ibx in range(ti2["x0a"] // SUB, 4):
                                av_done += 1
                                nc.tensor.matmul(
                                    pacc[:, ibx, :],
                                    e_pairs[pi2][:, h2,
                                                 ibx * SUB:(ibx + 1) * SUB],
                                    vw_sb[:, ti2["tau"], :],
                                    start=False, stop=(av_done == n_av))

                    for pi, (ia, ib_) in enumerate(pairs):
                        ta = taus[ia]
                        tb = taus[ib_] if ib_ is not None else None
                        nm = f"{bi}_{r}_{ta['tau']}"
                        ps_pr = ps_pool.tile([128, 2, 512], f32, tag="s",
                                             name=f"s_{nm}")
                        e_pr = sb_e.tile([128, 2, 512], f16, tag="e",
                                         name=f"e_{nm}")
                        ps_pairs[pi] = ps_pr
                        e_pairs[pi] = e_pr
                        for h, ti in enumerate((ta,) + ((tb,) if tb else ())):
                            nc.tensor.matmul(
                                ps_pr[:, h, ti["x0s"]:],
                                k_sb[:, ti["tau"] * JW:(ti["tau"] + 1) * JW],
                                q_sb[:, r * IW + ti["x0s"]:(r + 1) * IW],
                                start=True, stop=True)
                        kind, p_ett = gr.pick_pair(ta, tb)
                        x0a_, x0b_ = ta["x0s"], (tb["x0s"] if tb else 0)
                        if tb and kind == "AA":
                            xm = min(x0a_, x0b_)
                            act_exp(e_pr[:, :, xm:], ps_pr[:, :, xm:])
                            act_mask(e_pr[:, 0, :], ta, m_sb)
                            act_mask(e_pr[:, 1, :], tb, m_sb)
                        elif tb and kind == "DD":
                            xm = min(x0a_, x0b_)
                            eh = sb_eh.tile([128, 2, IW], f16, tag="eh",
                                            name=f"eh_{nm}")
                            clean = (ta["w1"] == x0a_ and tb["w1"] == x0b_
                                     and x0a_ == x0b_)
                            if clean:
                                nc.vector.tensor_scalar(
                                    out=eh[:, :, xm:].bitcast(i16),
                                    in0=ps_pr[:, :, xm:],
                                    scalar1=0.0, scalar2=None,
                                    op0=mybir.AluOpType.max)
                            else:
                                trick_op1(eh[:, 0, x0a_:].bitcast(i16),
                                          ps_pr[:, 0, x0a_:], ta, m_sb)
                                trick_op1(eh[:, 1, x0b_:].bitcast(i16),
                                          ps_pr[:, 1, x0b_:], tb, m_sb)
                            fix_chain(eh[:, :, xm:], e_pr[:, :, xm:], nm, p_ett)
                        else:
                            for h, ti in enumerate(
                                    (ta,) + ((tb,) if tb else ())):
                                eng = kind[h]
                                x0 = ti["x0s"]
                                if eng == "A":
                                    act_exp(e_pr[:, h, x0:],
                                            ps_pr[:, h, x0:])
                                    act_mask(e_pr[:, h, :], ti, m_sb)
                                else:
                                    eh = sb_eh.tile([128, 2, IW], f16,
                                                    tag="eh",
                                                    name=f"eh_{nm}_{h}")
                                    trick_op1(eh[:, 0, x0:].bitcast(i16),
                                              ps_pr[:, h, x0:], ti, m_sb)
                                    fix_chain(eh[:, 0, x0:],
                                              e_pr[:, h, x0:],
                                              f"{nm}_{h}", p_ett)
                        if pi >= 3:
                            emit_av_pair(pi - 3)
                    for pi2 in range(max(0, len(pairs) - 3), len(pairs)):
                        emit_av_pair(pi2)

                    pending_norm.append((bi, r, pacc))
                    if len(pending_norm) > 1:
                        emit_norm(*pending_norm.pop(0))
            while pending_norm:
                emit_norm(*pending_norm.pop(0))
    nc.compile()
    return nc


def _host_prep(q, k, v, valid):
    t = np.clip(np.asarray(valid).astype(np.int64), 0, N)
    perm = np.argsort(t, axis=1, kind="stable")
    t_s = np.take_along_axis(t, perm, axis=1)
    q_s = np.take_along_axis(np.asarray(q, np.float32), perm[..., None],
                             axis=1)
    plan = _classify(t_s)

    qT = np.empty((B, DV, N), np.float16)
    qT[:, 0:D] = np.swapaxes(q_s * SQ_A, 1, 2).astype(np.float16)
    qT[:, D] = 128.0
    kT = np.empty((B, DV, N), np.float16)
    kT[:, 0:D] = np.swapaxes(np.asarray(k, np.float32) * SQ_A, 1, 2
                             ).astype(np.float16)
    kT[:, D] = 120.0

    v32 = np.asarray(v, np.float32)
    vwt = np.empty((B, 128, NJ, DV), np.float16)
    vwt[:, :, :, 0:D] = np.swapaxes(
        v32.reshape(B, NJ, 128, D), 1, 2).astype(np.float16)
    vwt[:, :, :, D] = 1.0

    ss = np.zeros((B, N + 1, D), np.float64)
    ss[:, :-1] = np.cumsum(v32[:, ::-1, :].astype(np.float64),
                           axis=1)[:, ::-1, :]
    ssg = np.take_along_axis(ss, t_s[..., None], axis=1)   # [B, N, D]
    cnt = (N - t_s).astype(np.float64)                     # [B, N]
    corr = np.empty((B, N, DV), np.float64)
    corr[:, :, 0:D] = ssg * E6
    corr[:, :, D] = cnt * E6
    corrt = np.ascontiguousarray(
        corr.reshape(B, NI, 4, 128, DV).transpose(0, 3, 1, 2, 4)
    ).astype(np.float16)

    m16v = np.zeros((B, 128, plan.m16_w), np.float16)
    jj = np.arange(128)
    for r in range(NI):
        for ti in plan.taus[r]:
            if ti["m_off"] is None:
                continue
            x0, w1, tau = ti["x0s"], ti["w1"], ti["tau"]
            tloc = t_s[:, r * IW + x0: r * IW + w1]          # [B, w]
            mloc = tloc[:, None, :] > (JW * tau + jj)[None, :, None]
            m16v[:, :, ti["m_off"]:ti["m_off"] + (w1 - x0)] = mloc
    return plan, perm, qT, kT, vwt, corrt, m16v


LAST = {}


def kernel(q, k, v, valid, _trace=False):
    plan, perm, qT, kT, vwt, corrt, m16v = _host_prep(q, k, v, valid)
    nc = _build_program(plan)

    in_maps = []
    for c in range(NCORES):
        sl = slice(c * NB, (c + 1) * NB)
        in_maps.append({
            "qT": np.ascontiguousarray(qT[sl]),
            "kT": np.ascontiguousarray(kT[sl]),
            "vw": np.ascontiguousarray(vwt[sl]),
            "corr": np.ascontiguousarray(corrt[sl]),
            "m16": np.ascontiguousarray(m16v[sl]),
        })
    res = run_bass_kernel_spmd(nc, in_maps, list(range(NCORES)),
                               trace=_trace)
    LAST["res"] = res
    LAST["nc"] = nc

    out = np.empty((B, N, D), np.float32)
    for c in range(NCORES):
        o = res.results[c]["out"]          # [NB, 128, NI, 4, D] fp16
        for bi in range(NB):
            b = c * NB + bi
            o_sorted = o[bi].transpose(1, 2, 0, 3).reshape(N, D)
            out[b, perm[b]] = o_sorted.astype(np.float32)
    return out


# revision 24
# speedup vs baseline: 1.0042x; 1.0042x over previous
"""Bass/Trainium2 kernel for masked dot-product attention.

Math (per batch b):
  scores = q @ k^T / sqrt(D); masked positions (j >= valid[i]) -> 1e-6
  weights = softmax(scores, -1); out = weights @ v

Strategy (v3):
  - Shard batch dim B=16 across 8 cores (2 batches/core), SPMD program.
  - Host: sort rows of each batch by valid[i] -> monotone mask staircase;
    fully-masked (i,j)-tiles are skipped; their exact contribution
    exp(1e-6)*(suffix sums of v) is added via a host-built correction
    tensor (identity matmul into the same accumulator).
  - Device: S^T tiles [j=128 part, i<=512 free] on PE in fp16, with q
    pre-scaled by sqrt(A), k by sqrt(A) (A = 1024*0.125*log2 e) and a
    65th contraction row adding B = 15360 = 1024*15: the psum value IS
    the fp16 bit pattern of e^{s/8} (Schraudolph).  exp splits between:
      * ACT: exact exp (scale/bias fold A,B away), fp16 out
      * DVE: convert-to-int16 (round-to-nearest) + a 3-op quadratic
        mantissa correction, all in 2x/4x-eligible 16-bit ops.
    The mask staircase is a {1,0} fp16 multiply (fused into the DVE
    path's first op; a separate cheap multiply after ACT tiles).
  - AV: out[i-subtile 128, 65] accumulates per j-tile with E stationary
    (moving dim 65 incl. ones column -> softmax denominator for free).
  - Normalize: DVE reciprocal of z + broadcast multiply -> fp16 out.
  - GPSIMD cannot touch PSUM; it only zero-fills e-tile gap rows.
  - PE p-state warmup: 8 throwaway matmuls during the input-DMA window.
"""

import numpy as np

import concourse.bass as bass
import concourse.tile as tile
import concourse.mybir as mybir
from concourse import bacc
from concourse.bass_utils import run_bass_kernel_spmd
from concourse.masks import make_identity

B, N, D = 16, 2048, 64
NCORES = 8
NB = B // NCORES          # batches per core
IW = 512                  # i-range width
NI = N // IW              # 4 i-ranges
JW = 128                  # j tile width
NJ = N // JW              # 16 j tiles
SUB = 128                 # AV i-subtile
DV = D + 1

f16 = mybir.dt.float16
f32 = mybir.dt.float32
i16 = mybir.dt.int16

A_TRICK = 184.664955          # 1024 * 0.125 * log2(e)
SQ_A = float(np.sqrt(A_TRICK))
B_OFF = 15360.0               # 1024 * 15 (fp16 exponent bias)
SC_ACT = 0.125 / A_TRICK
E6 = float(np.exp(np.float32(1e-6)))

# cotrick: the product eh*eh2 (eh2 = bitcast(30720-bits)) equals the
# interp-error hump h(f) = (1+f)(2-f)/2; a linear map a+b*h approximates
# the exact correction c(f) = 2^f/(1+f) to +-0.6%. No common factor.
_ff = np.linspace(0.0, 1.0, 2001)
_cf = 2.0**_ff / (1.0 + _ff)
_hf = (1.0 + _ff) * (2.0 - _ff) / 2.0
CT_B, CT_A = [float(x) for x in np.polyfit(_hf, _cf, 1)]
LN_C2 = 0.0

LOOKAHEAD = 3
ACT_W = 0.75


class Plan:
    def __init__(self):
        self.taus = [[] for _ in range(NI)]   # per r: list of tau dicts
        self.m16_w = 0


def _classify(t_sorted):
    plan = Plan()
    off = 0
    for r in range(NI):
        tw = t_sorted[:, r * IW:(r + 1) * IW]  # [B, IW] sorted ascending
        for tau in range(NJ):
            jlo, jhi = JW * tau, JW * (tau + 1)
            n_le = (tw <= jlo).sum(axis=1)
            n_lt = (tw < jhi).sum(axis=1)
            lo = int(n_le.min())
            if lo >= IW:
                break
            mhi = int(n_lt.max())
            x0a = lo & ~127          # AV-subtile / S / exp start
            x0s = x0a
            w1 = max(mhi, x0s)
            ti = {
                "tau": tau, "lo": lo, "x0s": x0s, "x0a": x0a, "w1": w1,
                "m_off": None,
            }
            if w1 > x0s:
                ti["m_off"] = off
                off += w1 - x0s
            plan.taus[r].append(ti)
    plan.m16_w = max(off, 16)
    plan.m16_r = []
    for r in range(NI):
        offs = [ti["m_off"] for ti in plan.taus[r] if ti["m_off"] is not None]
        ws = [ti["w1"] - ti["x0s"] for ti in plan.taus[r]
              if ti["m_off"] is not None]
        if offs:
            plan.m16_r.append((min(offs), max(o + w for o, w in zip(offs, ws))))
        else:
            plan.m16_r.append(None)
    return plan


class _Greedy:
    """Pair-level balance between ACT (exact exp), DVE (cotrick) and
    GPSIMD (final multiply offload, SBUF-only)."""

    def __init__(self, act_w=1.0):
        self.load = {"ACT": 2600.0, "DVE": 600.0, "POOL": 800.0}
        self.act_w = act_w

    @staticmethod
    def c_act1(n):
        return (n + 222) * 0.8333

    @staticmethod
    def c_act2(n):
        return (2 * n + 222) * 0.8333

    @staticmethod
    def c_dve1(n, pool_ett):
        c = ((n + 120) + n + 3 * 58) * 1.0417
        if not pool_ett:
            c += (0.5 * n + 58) * 1.0417
        return c

    @staticmethod
    def c_dve2(n, clean, pool_ett):
        op1 = (2 * n + 120) if clean else 2 * (n + 120)
        c = (op1 + 2.0 * n + 3 * 58) * 1.0417
        if not pool_ett:
            c += (n + 58) * 1.0417
        return c

    @staticmethod
    def c_pool_ett(n2):
        return (n2 / 0.42) * 0.8333 + 95.0 + 1e9  # disabled: latency hurts

    def _mx(self, ca, cd, cp):
        return max(self.load["ACT"] + ca, self.load["DVE"] + cd,
                   self.load["POOL"] + cp)

    def pick_pair(self, ta, tb):
        """Returns (kind, pool_ett)."""
        na = IW - ta["x0s"]
        wba = (ta["w1"] - ta["x0s"])
        mska = (0.5 * wba + 58) * 1.0417 if wba else 0.0
        if tb is None:
            opts = {
                ("AA", False): (self.c_act1(na) * self.act_w, mska, 0.0),
                ("DD", False): (0.0, self.c_dve1(na, False), 0.0),
                ("DD", True): (0.0, self.c_dve1(na, True),
                               self.c_pool_ett(na)),
            }
        else:
            nb = IW - tb["x0s"]
            wbb = (tb["w1"] - tb["x0s"])
            mskb = (0.5 * wbb + 58) * 1.0417 if wbb else 0.0
            nm = IW - min(ta["x0s"], tb["x0s"])
            clean = (wba == 0 and wbb == 0 and ta["x0s"] == tb["x0s"])
            opts = {
                ("AA", False): (self.c_act2(nm) * self.act_w,
                                mska + mskb, 0.0),
                ("DD", False): (0.0, self.c_dve2(nm, clean, False), 0.0),
                ("DD", True): (0.0, self.c_dve2(nm, clean, True),
                               self.c_pool_ett(2 * nm)),
                ("AD", False): (self.c_act1(na) * self.act_w,
                                mska + self.c_dve1(nb, False), 0.0),
                ("DA", False): (self.c_act1(nb) * self.act_w,
                                mskb + self.c_dve1(na, False), 0.0),
            }
        best, bc = None, None
        for key, (ca, cd, cp) in opts.items():
            m = self._mx(ca, cd, cp)
            if bc is None or m < bc:
                best, bc = key, m
        ca, cd, cp = opts[best]
        self.load["ACT"] += ca
        self.load["DVE"] += cd
        self.load["POOL"] += cp
        return best

    def pick_norm(self):
        self.load["DVE"] += 129.0
        act_c = 4 * (64 + 222) * 0.8333 * self.act_w
        dve_c = (256 + 120) * 1.0417
        if self.load["ACT"] + act_c <= self.load["DVE"] + dve_c:
            self.load["ACT"] += act_c
            return "ACT"
        self.load["DVE"] += dve_c
        return "DVE"


def _build_program(plan):
    nc = bacc.Bacc("TRN2", target_bir_lowering=False, debug=False)

    qT = nc.dram_tensor("qT", [NB, DV, N], f16, kind="ExternalInput").ap()
    kT = nc.dram_tensor("kT", [NB, DV, N], f16, kind="ExternalInput").ap()
    vw = nc.dram_tensor("vw", [NB, 128, NJ, DV], f16, kind="ExternalInput").ap()
    corr = nc.dram_tensor("corr", [NB, 128, NI, 4, DV], f16,
                          kind="ExternalInput").ap()
    m16 = nc.dram_tensor("m16", [NB, 128, plan.m16_w], f16,
                         kind="ExternalInput").ap()
    out = nc.dram_tensor("out", [NB, 128, NI, 4, D], f16,
                         kind="ExternalOutput").ap()

    gr = _Greedy(act_w=ACT_W)

    with tile.TileContext(nc, trace_sim=False) as tc:
        with (
            tc.tile_pool(name="consts", bufs=1) as consts,
            tc.tile_pool(name="sb_qk", bufs=2) as sb_qk,
            tc.tile_pool(name="sb_vc", bufs=2) as sb_vc,
            tc.tile_pool(name="sb_e", bufs=8) as sb_e,
            tc.tile_pool(name="sb_eh", bufs=5) as sb_eh,
            tc.tile_pool(name="sb_t", bufs=5) as sb_t,
            tc.tile_pool(name="sb_o", bufs=3) as sb_o,
            tc.tile_pool(name="sb_z", bufs=3) as sb_z,
            tc.tile_pool(name="ps_pool", bufs=3, space="PSUM") as ps_pool,
            tc.tile_pool(name="ps_acc", bufs=2, space="PSUM") as ps_acc,
        ):
            ident = consts.tile([128, 128], f32)
            make_identity(nc, ident)
            identh = consts.tile([128, 128], f16)
            nc.vector.tensor_copy(identh, ident)
            bias_t = consts.tile([128, 1], f32)
            nc.vector.memset(bias_t, float(-B_OFF * SC_ACT + LN_C2))
            wsrc = consts.tile([128, 512], f16)
            nc.vector.memset(wsrc, 0.125)

            # p-state warmup (small moving dim: don't delay first S)
            for wu in range(6):
                ps_w = ps_pool.tile([128, 2, 512], f32, tag="s",
                                    name=f"wu{wu}")
                nc.tensor.matmul(ps_w[:, 0, 0:128], identh,
                                 wsrc[:, 0:128], start=True, stop=True)

            # input DMAs for both batches up-front (sync/SP queue)
            ins = []
            for bi in range(NB):
                q_sb = sb_qk.tile([DV, N], f16, tag="q")
                k_sb = sb_qk.tile([DV, N], f16, tag="k")
                m_sb = sb_vc.tile([128, plan.m16_w], f16, tag="m16")
                nc.sync.dma_start(out=q_sb, in_=qT[bi])
                nc.sync.dma_start(out=k_sb[:, 0:N // 2],
                                  in_=kT[bi][:, 0:N // 2])
                nc.sync.dma_start(out=k_sb[:, N // 2:],
                                  in_=kT[bi][:, N // 2:])
                corr_sb = sb_vc.tile([128, NI, 4, DV], f16, tag="corr")
                nc.sync.dma_start(out=corr_sb, in_=corr[bi])
                vw_sb = sb_vc.tile([128, NJ, DV], f16, tag="vw")
                nc.sync.dma_start(out=vw_sb, in_=vw[bi])
                nc.sync.dma_start(out=m_sb, in_=m16[bi])
                ins.append((q_sb, k_sb, vw_sb, corr_sb, m_sb))

            RING = 6

            def trick_op1(dst_i16, ps_ap, ti, m_sb):
                """Schraudolph op1 for one tau half into dst (int16 view)."""
                x0, w1 = ti["x0s"], ti["w1"]
                if w1 > x0:
                    mo = ti["m_off"]
                    nc.vector.scalar_tensor_tensor(
                        out=dst_i16, in0=ps_ap, scalar=0.0,
                        in1=m_sb[:, mo:mo + (IW - x0)],
                        op0=mybir.AluOpType.max, op1=mybir.AluOpType.mult)
                else:
                    nc.vector.tensor_scalar(
                        out=dst_i16, in0=ps_ap, scalar1=0.0, scalar2=None,
                        op0=mybir.AluOpType.max)

            def fix_chain(eh_ap, e_ap, nm, pool_ett=False):
                """cotrick fix: e = (CT_A + CT_B*(eh*eh2))*eh."""
                fb = sb_t.tile(list(eh_ap.shape), f16, tag="fb",
                               name=f"fb_{nm}")
                nc.vector.tensor_scalar(
                    out=fb.bitcast(i16), in0=eh_ap.bitcast(i16),
                    scalar1=-1.0, scalar2=30720.0,
                    op0=mybir.AluOpType.mult, op1=mybir.AluOpType.add)
                nc.vector.tensor_tensor(
                    out=fb, in0=eh_ap, in1=fb, op=mybir.AluOpType.mult)
                nc.vector.tensor_scalar(
                    out=fb, in0=fb, scalar1=float(CT_B), scalar2=float(CT_A),
                    op0=mybir.AluOpType.mult, op1=mybir.AluOpType.add)
                eng = nc.gpsimd if pool_ett else nc.vector
                eng.tensor_tensor(
                    out=e_ap, in0=fb, in1=eh_ap, op=mybir.AluOpType.mult)

            def act_exp(e_ap, ps_ap):
                nc.scalar.activation(
                    e_ap, ps_ap, mybir.ActivationFunctionType.Exp,
                    bias=bias_t[:, 0:1], scale=float(SC_ACT))

            def act_mask(e_t_ap, ti, m_sb):
                x0, w1 = ti["x0s"], ti["w1"]
                if w1 > x0:
                    mo = ti["m_off"]
                    nc.vector.tensor_tensor(
                        out=e_t_ap[:, x0:w1], in0=e_t_ap[:, x0:w1],
                        in1=m_sb[:, mo:mo + (w1 - x0)],
                        op=mybir.AluOpType.mult)

            pending_norm = []

            def emit_norm(bi3, r3, pacc3):
                zinv = sb_z.tile([128, 4], f32, tag="z",
                                 name=f"z_{bi3}_{r3}")
                nc.vector.reciprocal(zinv, pacc3[:, :, D])
                osb = sb_o.tile([128, 4, D], f16, tag="o",
                                name=f"o_{bi3}_{r3}")
                if gr.pick_norm() == "ACT":
                    for ib2 in range(4):
                        nc.scalar.activation(
                            osb[:, ib2, :], pacc3[:, ib2, 0:D],
                            mybir.ActivationFunctionType.Copy,
                            scale=zinv[:, ib2:ib2 + 1])
                else:
                    nc.vector.tensor_tensor(
                        out=osb, in0=pacc3[:, :, 0:D],
                        in1=zinv.unsqueeze(2).broadcast_to([128, 4, D]),
                        op=mybir.AluOpType.mult)
                nc.sync.dma_start(out=out[bi3][:, r3], in_=osb)

            for bi in range(NB):
                q_sb, k_sb, vw_sb, corr_sb, m_sb = ins[bi]
                for r in range(NI - 1, -1, -1):
                    taus = plan.taus[r]
                    pacc = ps_acc.tile([128, 4, DV], f32, tag="acc",
                                       name=f"acc_{bi}_{r}")
                    nc.tensor.matmul(
                        pacc, identh, corr_sb[:, r],
                        start=True, stop=False)

                    n_av = sum(4 - ti["x0a"] // SUB for ti in taus)
                    av_done = 0

                    pairs = [(i, i + 1 if i + 1 < len(taus) else None)
                             for i in range(0, len(taus), 2)]
                    ps_pairs = [None] * len(pairs)
                    e_pairs = [None] * len(pairs)

                    def emit_av_pair(pi2):
                        nonlocal av_done
                        ia2, ib2 = pairs[pi2]
                        for h2, idx2 in enumerate(
                                (ia2,) + ((ib2,) if ib2 is not None else ())):
                            ti2 = taus[idx2]
                            for ibx in range(ti2["x0a"] // SUB, 4):
                                av_done += 1
                                nc.tensor.matmul(
                                    pacc[:, ibx, :],
                                    e_pairs[pi2][:, h2,
                                                 ibx * SUB:(ibx + 1) * SUB],
                                    vw_sb[:, ti2["tau"], :],
                                    start=False, stop=(av_done == n_av))

                    for pi, (ia, ib_) in enumerate(pairs):
                        ta = taus[ia]
                        tb = taus[ib_] if ib_ is not None else None
                        nm = f"{bi}_{r}_{ta['tau']}"
                        ps_pr = ps_pool.tile([128, 2, 512], f32, tag="s",
                                             name=f"s_{nm}")
                        e_pr = sb_e.tile([128, 2, 512], f16, tag="e",
                                         name=f"e_{nm}")
                        ps_pairs[pi] = ps_pr
                        e_pairs[pi] = e_pr
                        for h, ti in enumerate((ta,) + ((tb,) if tb else ())):
                            nc.tensor.matmul(
                                ps_pr[:, h, ti["x0s"]:],
                                k_sb[:, ti["tau"] * JW:(ti["tau"] + 1) * JW],
                                q_sb[:, r * IW + ti["x0s"]:(r + 1) * IW],
                                start=True, stop=True)
                        if pi == 1 and pending_norm:
                            emit_norm(*pending_norm.pop(0))
                        kind, p_ett = gr.pick_pair(ta, tb)
                        x0a_, x0b_ = ta["x0s"], (tb["x0s"] if tb else 0)
                        if tb and kind == "AA":
                            xm = min(x0a_, x0b_)
                            act_exp(e_pr[:, :, xm:], ps_pr[:, :, xm:])
                            act_mask(e_pr[:, 0, :], ta, m_sb)
                            act_mask(e_pr[:, 1, :], tb, m_sb)
                        elif tb and kind == "DD":
                            xm = min(x0a_, x0b_)
                            eh = sb_eh.tile([128, 2, IW], f16, tag="eh",
                                            name=f"eh_{nm}")
                            clean = (ta["w1"] == x0a_ and tb["w1"] == x0b_
                                     and x0a_ == x0b_)
                            if clean:
                                nc.vector.tensor_scalar(
                                    out=eh[:, :, xm:].bitcast(i16),
                                    in0=ps_pr[:, :, xm:],
                                    scalar1=0.0, scalar2=None,
                                    op0=mybir.AluOpType.max)
                            else:
                                trick_op1(eh[:, 0, x0a_:].bitcast(i16),
                                          ps_pr[:, 0, x0a_:], ta, m_sb)
                                trick_op1(eh[:, 1, x0b_:].bitcast(i16),
                                          ps_pr[:, 1, x0b_:], tb, m_sb)
                            fix_chain(eh[:, :, xm:], e_pr[:, :, xm:], nm, p_ett)
                        else:
                            for h, ti in enumerate(
                                    (ta,) + ((tb,) if tb else ())):
                                eng = kind[h]
                                x0 = ti["x0s"]
                                if eng == "A":
                                    act_exp(e_pr[:, h, x0:],
                                            ps_pr[:, h, x0:])
                                    act_mask(e_pr[:, h, :], ti, m_sb)
                                else:
                                    eh = sb_eh.tile([128, 2, IW], f16,
                                                    tag="eh",
                                                    name=f"eh_{nm}_{h}")
                                    trick_op1(eh[:, 0, x0:].bitcast(i16),
                                              ps_pr[:, h, x0:], ti, m_sb)
                                    fix_chain(eh[:, 0, x0:],
                                              e_pr[:, h, x0:],
                                              f"{nm}_{h}", p_ett)
                        if pi >= 3:
                            emit_av_pair(pi - 3)
                    for pi2 in range(max(0, len(pairs) - 3), len(pairs)):
                        emit_av_pair(pi2)

                    pending_norm.append((bi, r, pacc))
            while pending_norm:
                emit_norm(*pending_norm.pop(0))
    nc.compile()
    return nc


def _host_prep(q, k, v, valid):
    t = np.clip(np.asarray(valid).astype(np.int64), 0, N)
    perm = np.argsort(t, axis=1, kind="stable")
    t_s = np.take_along_axis(t, perm, axis=1)
    q_s = np.take_along_axis(np.asarray(q, np.float32), perm[..., None],
                             axis=1)
    plan = _classify(t_s)

    qT = np.empty((B, DV, N), np.float16)
    qT[:, 0:D] = np.swapaxes(q_s * SQ_A, 1, 2).astype(np.float16)
    qT[:, D] = 128.0
    kT = np.empty((B, DV, N), np.float16)
    kT[:, 0:D] = np.swapaxes(np.asarray(k, np.float32) * SQ_A, 1, 2
                             ).astype(np.float16)
    kT[:, D] = 120.0

    v32 = np.asarray(v, np.float32)
    vwt = np.empty((B, 128, NJ, DV), np.float16)
    vwt[:, :, :, 0:D] = np.swapaxes(
        v32.reshape(B, NJ, 128, D), 1, 2).astype(np.float16)
    vwt[:, :, :, D] = 1.0

    ss = np.zeros((B, N + 1, D), np.float64)
    ss[:, :-1] = np.cumsum(v32[:, ::-1, :].astype(np.float64),
                           axis=1)[:, ::-1, :]
    ssg = np.take_along_axis(ss, t_s[..., None], axis=1)   # [B, N, D]
    cnt = (N - t_s).astype(np.float64)                     # [B, N]
    corr = np.empty((B, N, DV), np.float64)
    corr[:, :, 0:D] = ssg * E6
    corr[:, :, D] = cnt * E6
    corrt = np.ascontiguousarray(
        corr.reshape(B, NI, 4, 128, DV).transpose(0, 3, 1, 2, 4)
    ).astype(np.float16)

    m16v = np.zeros((B, 128, plan.m16_w), np.float16)
    jj = np.arange(128)
    for r in range(NI):
        for ti in plan.taus[r]:
            if ti["m_off"] is None:
                continue
            x0, w1, tau = ti["x0s"], ti["w1"], ti["tau"]
            tloc = t_s[:, r * IW + x0: r * IW + w1]          # [B, w]
            mloc = tloc[:, None, :] > (JW * tau + jj)[None, :, None]
            m16v[:, :, ti["m_off"]:ti["m_off"] + (w1 - x0)] = mloc
    return plan, perm, qT, kT, vwt, corrt, m16v


LAST = {}


def kernel(q, k, v, valid, _trace=False):
    plan, perm, qT, kT, vwt, corrt, m16v = _host_prep(q, k, v, valid)
    nc = _build_program(plan)

    in_maps = []
    for c in range(NCORES):
        sl = slice(c * NB, (c + 1) * NB)
        in_maps.append({
            "qT": np.ascontiguousarray(qT[sl]),
            "kT": np.ascontiguousarray(kT[sl]),
            "vw": np.ascontiguousarray(vwt[sl]),
            "corr": np.ascontiguousarray(corrt[sl]),
            "m16": np.ascontiguousarray(m16v[sl]),
        })
    res = run_bass_kernel_spmd(nc, in_maps, list(range(NCORES)),
                               trace=_trace)
    LAST["res"] = res
    LAST["nc"] = nc

    out = np.empty((B, N, D), np.float32)
    for c in range(NCORES):
        o = res.results[c]["out"]          # [NB, 128, NI, 4, D] fp16
        for bi in range(NB):
            b = c * NB + bi
            o_sorted = o[bi].transpose(1, 2, 0, 3).reshape(N, D)
            out[b, perm[b]] = o_sorted.astype(np.float32)
    return out
